# revision 4
# baseline (speedup 1.0000x reference)
"""Fused transformer block (LN-over-sequence + causal MHA + LN + MLP, residuals)
for Trainium2, distributed over 8 NeuronCores.

Distribution:
  Phase A (attention): tensor-parallel over heads -- core c owns heads (2c, 2c+1)
    = output channels [128c, 128c+128). Everything is kept channel-major
    ("T" layouts: [channels, tokens]) so the sequence-dim LayerNorms reduce
    along the free axis and no transposes are needed.
    LN1 is folded into the QKV weights (scale weight rows by g1/std, add a
    bias column W.T @ b) so the 16MB x tensor is never rewritten.
    Softmax is computed transposed (weiT[s,t]) without max-subtraction
    (logits are ~N(0, 1/16), bounded far below exp overflow); the
    denominator comes from an appended ones-column in V (attn row 64).
  Re-shard: one AllToAll converts channel-sharding -> token-sharding:
    core c sends x1T[own 128 channels, tokens of core j] (+ LN2 scale/bias
    columns) to core j.
  Phase B (MLP): token-parallel -- each core runs the full 1024->4096->1024
    MLP for its 512 tokens, channel-major, with the residual add fused.

All matmuls run in float32r (TF32-like, 4x fp32 throughput, ~1.5e-4 rel err).
"""

import numpy as np

import concourse.bass as bass
import concourse.mybir as mybir
import concourse.tile as tile
from concourse import bacc
from concourse.bass_utils import run_bass_kernel_spmd

B, T, C, H, D = 2, 2048, 1024, 16, 64
NCORES = 8
CT = C // 128  # 8 channel tiles
TS = 512  # token slice width
NTS = T // TS  # 4
NST = T // 128  # 16 s-tiles
HID = 4 * C
EPS = 1e-5
VCORR = T / (T - 1.0)  # unbiased-variance correction (torch.var ddof=1)
SCALE = C**-0.5  # attention logit scale = 1/32

F32 = mybir.dt.float32
F32R = mybir.dt.float32r
AF = mybir.ActivationFunctionType
ALU = mybir.AluOpType

_CACHE: dict = {}


def build():
    nc = bacc.Bacc(None, target_bir_lowering=False, debug=False, num_devices=NCORES)

    def din(name, shape, dt=F32):
        return nc.dram_tensor(name, shape, dt, kind="ExternalInput").ap()

    xT_d = din("xT", [B, CT, 128, T], F32R)  # x channel-major (shared, rounded)
    xown_d = din("xown", [B, 2, 64, T], F32)  # core's own channels, raw fp32
    wqq_d = din("wqq", [CT, 128, 128], F32)  # [Wq_h0 | Wq_h1] row-tiled
    wkk_d = din("wkk", [CT, 128, 128], F32)
    wvv_d = din("wvv", [CT, 128, 128], F32)
    g1_d = din("g1c", [128, CT], F32)  # per-channel LN1 gain, partition-major
    be1_d = din("be1c", [128, CT], F32)
    g2_d = din("g2o", [2, 64, 1], F32)  # LN2 gain for own channels, per head-half
    be2_d = din("be2o", [2, 64, 1], F32)
    b1_d = din("b1c", [128, HID // 128], F32)  # MLP biases, partition-major
    b2_d = din("b2c", [128, CT], F32)
    dm_d = din("dmask", [4, 128, TS], F32R)  # causal masks for diag-crossing slice
    id_d = din("identr", [128, 128], F32R)
    onr_d = din("onesrow", [1, 64], F32R)
    onc_d = din("onescol", [128, 1], F32R)
    w1_d = din("w1t", [CT, 8, 128, TS], F32R)  # W1 tiled [c-tile, hid/512, p, f]
    w2_d = din("w2t", [HID // 128, 2, 128, TS], F32R)  # W2 tiled [hid-tile, emb/512, p, f]

    outT_d = nc.dram_tensor("outT", [CT, 128, TS], F32, kind="ExternalOutput").ap()

    payload = nc.dram_tensor("payload", [NCORES * 128, 516], F32).ap()
    gath = nc.dram_tensor("gath", [NCORES * 128, 516], F32).ap()

    with tile.TileContext(nc) as tc:
        with tc.tile_pool(name="cst", bufs=1) as cst:
            # constants needed across phases
            b1t = cst.tile([128, HID // 128], F32)
            nc.sync.dma_start(b1t[:], b1_d[:])
            b2t = cst.tile([128, CT], F32)
            nc.sync.dma_start(b2t[:], b2_d[:])

            # ---------------- PHASE A ----------------
            with (
                tc.tile_pool(name="ac", bufs=1) as ac,
                tc.tile_pool(name="xtp", bufs=9) as xtp,
                tc.tile_pool(name="wsp", bufs=1) as wsp,
                tc.tile_pool(name="qkp", bufs=1) as qkp,
                tc.tile_pool(name="vp", bufs=1) as vp,
                tc.tile_pool(name="weip", bufs=6) as weip,
                tc.tile_pool(name="xhp", bufs=1) as xhp,
                tc.tile_pool(name="sp", bufs=2) as sp,
                tc.tile_pool(name="psmm", bufs=2, space="PSUM") as psmm,
                tc.tile_pool(name="psat", bufs=1, space="PSUM") as psat,
                tc.tile_pool(name="psax", bufs=2, space="PSUM") as psax,
            ):
                g1t = ac.tile([128, CT], F32)
                nc.sync.dma_start(g1t[:], g1_d[:])
                be1t = ac.tile([128, CT], F32)
                nc.sync.dma_start(be1t[:], be1_d[:])
                g2t = [ac.tile([64, 1], F32, tag=f"g2_{hl}", name=f"g2_{hl}") for hl in range(2)]
                be2t = [ac.tile([64, 1], F32, tag=f"be2_{hl}", name=f"be2_{hl}") for hl in range(2)]
                for hl in range(2):
                    nc.sync.dma_start(g2t[hl][:], g2_d[hl])
                    nc.sync.dma_start(be2t[hl][:], be2_d[hl])
                dms = [ac.tile([128, TS], F32R, tag=f"dm{o}", name=f"dm{o}") for o in range(4)]
                for o in range(4):
                    nc.sync.dma_start(dms[o][:], dm_d[o])
                idn = ac.tile([128, 128], F32R)
                nc.sync.dma_start(idn[:], id_d[:])
                onr = ac.tile([1, 64], F32R)
                nc.sync.dma_start(onr[:], onr_d[:])
                onc = ac.tile([128, 1], F32R)
                nc.sync.dma_start(onc[:], onc_d[:])
                wraw = {}
                for pname, pd in (("qq", wqq_d), ("kk", wkk_d), ("vv", wvv_d)):
                    for ct in range(CT):
                        w = ac.tile([128, 128], F32, tag=f"wr_{pname}_{ct}")
                        nc.sync.dma_start(w[:], pd[ct])
                        wraw[(pname, ct)] = w

                xh = {}  # raw x (later x1) per (b, head-half), [64, T] fp32
                for b in range(B):
                    for hl in range(2):
                        t_ = xhp.tile([64, T], F32, tag=f"xh_{b}_{hl}")
                        nc.sync.dma_start(t_[:], xown_d[b, hl])
                        xh[(b, hl)] = t_

                for b in range(B):
                    # -- load xT tiles (fp32r, rounded on the fly)
                    xts = []
                    for ct in range(CT):
                        t_ = xtp.tile([128, T], F32R, tag="xt")
                        nc.sync.dma_start(t_[:], xT_d[b, ct])
                        xts.append(t_)

                    # -- LN1 stats -> per-channel scale s1 / bias bb1 columns
                    s1c, bb1c = [], []
                    for ct in range(CT):
                        st6 = sp.tile([128, 4, 6], F32, tag="st6")
                        for i in range(4):
                            nc.vector.bn_stats(
                                st6[:, i, :],
                                xts[ct][:, i * TS : (i + 1) * TS].bitcast(F32),
                            )
                        mv = sp.tile([128, 2], F32, tag="mv")
                        nc.vector.bn_aggr(mv[:], st6[:])
                        va = sp.tile([128, 1], F32, tag="va")
                        nc.vector.tensor_scalar(
                            out=va[:], in0=mv[:, 1:2], scalar1=VCORR, scalar2=EPS,
                            op0=ALU.mult, op1=ALU.add,
                        )
                        sq = sp.tile([128, 1], F32, tag="sq")
                        nc.scalar.activation(sq[:], va[:], AF.Sqrt)
                        rs = sp.tile([128, 1], F32, tag="rs")
                        with nc.allow_low_precision(reason="LN rstd"):
                            nc.vector.reciprocal(rs[:], sq[:])
                        s1 = sp.tile([128, 1], F32, tag=f"s1_{ct}")
                        nc.vector.tensor_mul(s1[:], g1t[:, ct : ct + 1], rs[:])
                        bb = sp.tile([128, 1], F32, tag=f"bb_{ct}")
                        nc.vector.tensor_mul(bb[:], mv[:, 0:1], s1[:])
                        nc.vector.tensor_sub(bb[:], be1t[:, ct : ct + 1], bb[:])
                        s1c.append(s1)
                        bb1c.append(bb)

                    # -- fold LN1 into weights: bias col = Wraw.T @ bb ; W' = s1 * Wraw
                    bcol = {}
                    ws = {}
                    for pname in ("qq", "kk", "vv"):
                        bp_ = psax.tile([128, 1], F32, tag="aux")
                        for ct in range(CT):
                            nc.tensor.matmul(
                                bp_[:], wraw[(pname, ct)][:], bb1c[ct][:],
                                start=(ct == 0), stop=(ct == CT - 1),
                            )
                        bc = sp.tile([128, 1], F32, tag=f"bc_{pname}")
                        nc.vector.tensor_copy(bc[:], bp_[:])
                        bcol[pname] = bc
                        for ct in range(CT):
                            w_ = wsp.tile([128, 128], F32R, tag=f"ws_{pname}_{ct}")
                            nc.vector.tensor_mul(
                                w_[:], wraw[(pname, ct)][:],
                                s1c[ct][:].broadcast_to((128, 128)),
                            )
                            ws[(pname, ct)] = w_

                    # -- QKV: packT streams [128, T] fp32r
                    packT = {}
                    for pname in ("qq", "kk", "vv"):
                        o_ = qkp.tile([128, T], F32R, tag=f"{pname}T")
                        for ts in range(NTS):
                            ps = psmm.tile([128, TS], F32, tag="mm")
                            for ct in range(CT):
                                nc.tensor.matmul(
                                    ps[:], ws[(pname, ct)][:],
                                    xts[ct][:, ts * TS : (ts + 1) * TS],
                                    start=(ct == 0), stop=(ct == CT - 1),
                                )
                            nc.scalar.activation(
                                o_[:, ts * TS : (ts + 1) * TS], ps[:], AF.Identity,
                                bias=bcol[pname][:],
                            )
                        packT[pname] = o_

                    # -- V transpose to token-major v_aug tiles [128, 65]
                    vh = {}
                    for st in range(NST):
                        pt_ = psax.tile([128, 128], F32R, tag="aux")
                        nc.tensor.transpose(
                            pt_[:], packT["vv"][:, st * 128 : (st + 1) * 128], idn[:]
                        )
                        for hl in range(2):
                            va_ = vp.tile([128, 65], F32R, tag=f"v_{hl}_{st}")
                            nc.vector.tensor_copy(
                                va_[:, 0:64], pt_[:, 64 * hl : 64 * hl + 64]
                            )
                            nc.vector.tensor_copy(va_[:, 64:65], onc[:])
                            vh[(hl, st)] = va_

                    # -- attention per head-half
                    for hl in range(2):
                        lo = 64 * hl
                        aps = [psat.tile([65, TS], F32, tag=f"at{ts}", name=f"at{ts}") for ts in range(NTS)]
                        for st in range(NST):
                            ts0 = st // 4
                            weis = {}
                            for ts in range(ts0, NTS):
                                wp = psmm.tile([128, TS], F32, tag="mm")
                                nc.tensor.matmul(
                                    wp[:],
                                    packT["kk"][lo : lo + 64, st * 128 : (st + 1) * 128],
                                    packT["qq"][lo : lo + 64, ts * TS : (ts + 1) * TS],
                                    start=True, stop=True,
                                )
                                we = weip.tile([128, TS], F32R, tag="wei")
                                nc.scalar.activation(we[:], wp[:], AF.Exp, scale=SCALE)
                                if ts == ts0:
                                    nc.vector.tensor_mul(we[:], we[:], dms[st % 4][:])
                                weis[ts] = we
                            for ts in range(ts0, NTS):
                                nc.tensor.matmul(
                                    aps[ts][:], vh[(hl, st)][:], weis[ts][:],
                                    start=(st == 0), stop=(st == 4 * ts + 3),
                                )
                            for ts in range(NTS):
                                if 4 * ts + 3 != st:
                                    continue
                                # attn complete for this t-slice: normalize + residual
                                rec = sp.tile([1, TS], F32R, tag="rec")
                                with nc.allow_low_precision(reason="softmax denom"):
                                    nc.vector.reciprocal(rec[:], aps[ts][64:65, :])
                                rbp = psax.tile([64, TS], F32, tag="aux")
                                nc.tensor.matmul(
                                    rbp[:], onr[:], rec[:], start=True, stop=True
                                )
                                rb = sp.tile([64, TS], F32, tag="rb")
                                nc.vector.tensor_copy(rb[:], rbp[:])
                                tmp = sp.tile([64, TS], F32, tag="tmp")
                                nc.vector.tensor_mul(tmp[:], aps[ts][0:64, :], rb[:])
                                xs = xh[(b, hl)][:, ts * TS : (ts + 1) * TS]
                                nc.vector.tensor_add(xs, xs, tmp[:])

                        # -- LN2 stats on x1 (now in xh) -> s2/b2 cols -> payload
                        st6 = sp.tile([64, 4, 6], F32, tag="st6b")
                        for i in range(4):
                            nc.vector.bn_stats(
                                st6[:, i, :], xh[(b, hl)][:, i * TS : (i + 1) * TS]
                            )
                        mv = sp.tile([64, 2], F32, tag="mv2")
                        nc.vector.bn_aggr(mv[:], st6[:])
                        va = sp.tile([64, 1], F32, tag="va2")
                        nc.vector.tensor_scalar(
                            out=va[:], in0=mv[:, 1:2], scalar1=VCORR, scalar2=EPS,
                            op0=ALU.mult, op1=ALU.add,
                        )
                        sq = sp.tile([64, 1], F32, tag="sq2")
                        nc.scalar.activation(sq[:], va[:], AF.Sqrt)
                        rs = sp.tile([64, 1], F32, tag="rs2")
                        with nc.allow_low_precision(reason="LN rstd"):
                            nc.vector.reciprocal(rs[:], sq[:])
                        s2 = sp.tile([64, 1], F32, tag="s2")
                        nc.vector.tensor_mul(s2[:], g2t[hl][:], rs[:])
                        b2_ = sp.tile([64, 1], F32, tag="b2_")
                        nc.vector.tensor_mul(b2_[:], mv[:, 0:1], s2[:])
                        nc.vector.tensor_sub(b2_[:], be2t[hl][:], b2_[:])

                        for jj in range(4):
                            j = 4 * b + jj
                            r0 = 128 * j + lo
                            nc.sync.dma_start(
                                payload[r0 : r0 + 64, 0:TS],
                                xh[(b, hl)][:, jj * TS : (jj + 1) * TS],
                            )
                            nc.sync.dma_start(payload[r0 : r0 + 64, 512:513], s2[:])
                            nc.sync.dma_start(payload[r0 : r0 + 64, 513:514], b2_[:])

            # ---------------- A2A re-shard ----------------
            nc.gpsimd.collective_compute(
                "AllToAll",
                ALU.bypass,
                ins=[payload[:]],
                outs=[gath[:]],
                replica_groups=[list(range(NCORES))],
            )

            # ---------------- PHASE B: MLP on own 512 tokens ----------------
            with (
                tc.tile_pool(name="bp", bufs=1) as bp,
                tc.tile_pool(name="psB", bufs=4, space="PSUM") as psB,
            ):
                x1g = []
                for i in range(CT):
                    t_ = bp.tile([128, 516], F32, tag=f"x1g{i}")
                    nc.sync.dma_start(t_[:], gath[128 * i : 128 * (i + 1), :])
                    x1g.append(t_)

                h1 = [bp.tile([128, TS], F32R, tag=f"h1_{j}", name=f"h1_{j}") for j in range(HID // 128)]

                with tc.tile_pool(name="w1p", bufs=20) as w1p, tc.tile_pool(
                    name="y2p", bufs=1
                ) as y2p:
                    y2 = []
                    for i in range(CT):
                        t_ = y2p.tile([128, TS], F32R, tag=f"y2{i}")
                        nc.scalar.activation(
                            t_[:], x1g[i][:, 0:TS], AF.Identity,
                            scale=x1g[i][:, 512:513], bias=x1g[i][:, 513:514],
                        )
                        y2.append(t_)
                    for jj in range(8):
                        w1sb = []
                        for i in range(CT):
                            w_ = w1p.tile([128, TS], F32R, tag="w1")
                            nc.sync.dma_start(w_[:], w1_d[i, jj])
                            w1sb.append(w_)
                        for j in range(4 * jj, 4 * jj + 4):
                            o = 128 * (j % 4)
                            ps = psB.tile([128, TS], F32, tag="bm")
                            for i in range(CT):
                                nc.tensor.matmul(
                                    ps[:], w1sb[i][:, o : o + 128], y2[i][:],
                                    start=(i == 0), stop=(i == CT - 1),
                                )
                            nc.scalar.activation(
                                h1[j][:], ps[:], AF.Relu, bias=b1t[:, j : j + 1]
                            )

                with tc.tile_pool(name="w2p", bufs=34) as w2p:
                    for kk in range(2):
                        w2sb = []
                        for j in range(HID // 128):
                            w_ = w2p.tile([128, TS], F32R, tag="w2")
                            nc.sync.dma_start(w_[:], w2_d[j, kk])
                            w2sb.append(w_)
                        for k in range(4 * kk, 4 * kk + 4):
                            o = 128 * (k % 4)
                            ps = psB.tile([128, TS], F32, tag="bm")
                            for j in range(HID // 128):
                                nc.tensor.matmul(
                                    ps[:], w2sb[j][:, o : o + 128], h1[j][:],
                                    start=(j == 0), stop=(j == HID // 128 - 1),
                                )
                            t1 = bp.tile([128, TS], F32, tag="ot")
                            nc.scalar.activation(
                                t1[:], ps[:], AF.Identity, bias=b2t[:, k : k + 1]
                            )
                            oo = bp.tile([128, TS], F32, tag="oo")
                            nc.vector.tensor_add(oo[:], t1[:], x1g[k][:, 0:TS])
                            nc.sync.dma_start(outT_d[k], oo[:])

    nc.compile()
    return nc


def _prep(inputs):
    x = np.ascontiguousarray(np.asarray(inputs["x"], np.float32))
    Wq = np.asarray(inputs["Wq"], np.float32)
    Wk = np.asarray(inputs["Wk"], np.float32)
    Wv = np.asarray(inputs["Wv"], np.float32)
    W1 = np.asarray(inputs["W1"], np.float32)
    W2 = np.asarray(inputs["W2"], np.float32)
    b1 = np.asarray(inputs["b1"], np.float32)
    b2 = np.asarray(inputs["b2"], np.float32)
    g1 = np.asarray(inputs["g1"], np.float32)
    be1 = np.asarray(inputs["be1"], np.float32)
    g2 = np.asarray(inputs["g2"], np.float32)
    be2 = np.asarray(inputs["be2"], np.float32)

    # channel-major x: xT[b, ct, p, t] = x[b, t, 128ct+p]
    xT = np.ascontiguousarray(x.reshape(B, T, CT, 128).transpose(0, 2, 3, 1))

    dmask = np.zeros((4, 128, TS), np.float32)
    t_idx = np.arange(TS)[None, :]
    p_idx = np.arange(128)[:, None]
    for o in range(4):
        dmask[o] = (t_idx >= 128 * o + p_idx).astype(np.float32)

    w1t = np.ascontiguousarray(W1.reshape(CT, 128, 8, TS).transpose(0, 2, 1, 3))
    w2t = np.ascontiguousarray(W2.reshape(HID // 128, 128, 2, TS).transpose(0, 2, 1, 3))

    shared = {
        "xT": xT,
        "g1c": np.ascontiguousarray(g1.reshape(CT, 128).T),
        "be1c": np.ascontiguousarray(be1.reshape(CT, 128).T),
        "b1c": np.ascontiguousarray(b1.reshape(HID // 128, 128).T),
        "b2c": np.ascontiguousarray(b2.reshape(CT, 128).T),
        "dmask": dmask,
        "identr": np.eye(128, dtype=np.float32),
        "onesrow": np.ones((1, 64), np.float32),
        "onescol": np.ones((128, 1), np.float32),
        "w1t": w1t,
        "w2t": w2t,
    }
    in_maps = []
    for c in range(NCORES):
        h0, h1_ = 2 * c, 2 * c + 1
        m = dict(shared)
        m["wqq"] = np.ascontiguousarray(
            np.concatenate([Wq[h0], Wq[h1_]], axis=1).reshape(CT, 128, 128)
        )
        m["wkk"] = np.ascontiguousarray(
            np.concatenate([Wk[h0], Wk[h1_]], axis=1).reshape(CT, 128, 128)
        )
        m["wvv"] = np.ascontiguousarray(
            np.concatenate([Wv[h0], Wv[h1_]], axis=1).reshape(CT, 128, 128)
        )
        m["xown"] = np.ascontiguousarray(xT[:, c].reshape(B, 2, 64, T))
        m["g2o"] = np.ascontiguousarray(
            g2.reshape(CT, 2, 64)[c][:, :, None]
        )
        m["be2o"] = np.ascontiguousarray(be2.reshape(CT, 2, 64)[c][:, :, None])
        in_maps.append(m)
    return in_maps


def kernel(**inputs) -> np.ndarray:
    if "nc" not in _CACHE:
        _CACHE["nc"] = build()
    nc = _CACHE["nc"]
    in_maps = _prep(inputs)
    res = run_bass_kernel_spmd(nc, in_maps, core_ids=list(range(NCORES)))
    out = np.empty((B, T, C), np.float32)
    for c in range(NCORES):
        b, t0 = c // 4, TS * (c % 4)
        oT = res.results[c]["outT"]  # [8, 128, 512]
        out[b, t0 : t0 + TS, :] = oT.transpose(2, 0, 1).reshape(TS, C)
    return out


# revision 5
# speedup vs baseline: 1.1116x; 1.1116x over previous
"""Fused transformer block (LN-over-sequence + causal MHA + LN + MLP, residuals)
for Trainium2, distributed over 8 NeuronCores.

Distribution:
  Phase A (attention): tensor-parallel over heads -- core c owns heads (2c, 2c+1)
    = output channels [128c, 128c+128). Everything is kept channel-major
    ("T" layouts: [channels, tokens]) so the sequence-dim LayerNorms reduce
    along the free axis and no transposes are needed.
    LN1 is folded into the QKV weights (scale weight rows by g1/std, add a
    bias column W.T @ b) so the 16MB x tensor is never rewritten.
    Softmax is computed transposed (weiT[s,t]) without max-subtraction
    (logits are ~N(0, 1/16), bounded far below exp overflow); the
    denominator comes from an appended ones-column in V (attn row 64).
    Causality: per s-tile only the t >= 128*st columns are computed
    (partial-width diagonal slices) + one [128,128] triangular mask.
  Re-shard: two AllToAlls (one per head-half, the first overlapped with the
    second half of attention) convert channel-sharding -> token-sharding:
    core c sends x1T[own channels, tokens of core j] (+ LN2 scale/bias
    columns) to core j.
  Phase B (MLP): token-parallel -- each core runs the full 1024->4096->1024
    MLP for its 512 tokens, channel-major, with the residual add fused.

All matmuls run in float32r (TF32-like, 4x fp32 throughput, ~1.5e-4 rel err).
"""

import numpy as np

import concourse.bass as bass
import concourse.mybir as mybir
import concourse.tile as tile
from concourse import bacc
from concourse.bass_utils import run_bass_kernel_spmd

B, T, C, H, D = 2, 2048, 1024, 16, 64
NCORES = 8
CT = C // 128  # 8 channel tiles
TS = 512  # token slice width
NTS = T // TS  # 4
NST = T // 128  # 16 s-tiles
HID = 4 * C
NJ = HID // 128  # 32 hidden tiles
EPS = 1e-5
VCORR = T / (T - 1.0)  # unbiased-variance correction (torch.var ddof=1)
SCALE = C**-0.5  # attention logit scale = 1/32

F32 = mybir.dt.float32
F32R = mybir.dt.float32r
AF = mybir.ActivationFunctionType
ALU = mybir.AluOpType

_CACHE: dict = {}


def build():
    nc = bacc.Bacc(None, target_bir_lowering=False, debug=False, num_devices=NCORES)

    def din(name, shape, dt=F32):
        return nc.dram_tensor(name, shape, dt, kind="ExternalInput").ap()

    xT_d = din("xT", [B, CT, 128, T], F32R)  # x channel-major (shared, rounded)
    xown_d = din("xown", [B, 2, 64, T], F32)  # core's own channels, raw fp32
    # packs laid out [128, 8*128]: col block ct holds rows 128ct..128ct+128
    wqq_d = din("wqq", [128, C], F32)
    wkk_d = din("wkk", [128, C], F32)
    wvv_d = din("wvv", [128, C], F32)
    g1_d = din("g1c", [128, CT], F32)
    be1_d = din("be1c", [128, CT], F32)
    g2_d = din("g2o", [2, 64, 1], F32)
    be2_d = din("be2o", [2, 64, 1], F32)
    b1_d = din("b1c", [128, NJ], F32)
    b2_d = din("b2c", [128, CT], F32)
    tm_d = din("trimask", [128, 128], F32R)  # tri: 1 if t_local >= s_local
    id_d = din("identr", [128, 128], F32R)
    onr_d = din("onesrow", [1, 64], F32R)
    onc_d = din("onescol", [128, 1], F32R)
    w1_d = din("w1t", [CT, 4, 128, 1024], F32R)  # W1 [c-tile, jj2, p, 2x512]
    w2_d = din("w2t", [NJ, 2, 128, TS], F32R)  # W2 [hid-tile, kk, p, 512]

    outT_d = nc.dram_tensor("outT", [CT, 128, TS], F32, kind="ExternalOutput").ap()

    payl = [
        nc.dram_tensor(f"payl{hl}", [NCORES * 64, 516], F32).ap() for hl in range(2)
    ]
    gath = [
        nc.dram_tensor(f"gath{hl}", [NCORES * 64, 516], F32).ap() for hl in range(2)
    ]

    with tile.TileContext(nc) as tc:
        with tc.tile_pool(name="cst", bufs=1) as cst:
            b1t = cst.tile([128, NJ], F32)
            nc.sync.dma_start(b1t[:], b1_d[:])
            b2t = cst.tile([128, CT], F32)
            nc.sync.dma_start(b2t[:], b2_d[:])

            # ---------------- PHASE A ----------------
            with (
                tc.tile_pool(name="ac", bufs=1) as ac,
                tc.tile_pool(name="xtp", bufs=9) as xtp,
                tc.tile_pool(name="wsp", bufs=1) as wsp,
                tc.tile_pool(name="qkp", bufs=1) as qkp,
                tc.tile_pool(name="vp", bufs=1) as vp,
                tc.tile_pool(name="weip", bufs=6) as weip,
                tc.tile_pool(name="xhp", bufs=1) as xhp,
                tc.tile_pool(name="sp", bufs=2) as sp,
                tc.tile_pool(name="psmm", bufs=2, space="PSUM") as psmm,
                tc.tile_pool(name="psat", bufs=1, space="PSUM") as psat,
                tc.tile_pool(name="psax", bufs=2, space="PSUM") as psax,
            ):
                g1t = ac.tile([128, CT], F32)
                nc.sync.dma_start(g1t[:], g1_d[:])
                be1t = ac.tile([128, CT], F32)
                nc.sync.dma_start(be1t[:], be1_d[:])
                g2t = [
                    ac.tile([64, 1], F32, name=f"g2_{hl}", tag=f"g2_{hl}")
                    for hl in range(2)
                ]
                be2t = [
                    ac.tile([64, 1], F32, name=f"be2_{hl}", tag=f"be2_{hl}")
                    for hl in range(2)
                ]
                for hl in range(2):
                    nc.sync.dma_start(g2t[hl][:], g2_d[hl])
                    nc.sync.dma_start(be2t[hl][:], be2_d[hl])
                tri = ac.tile([128, 128], F32R)
                nc.sync.dma_start(tri[:], tm_d[:])
                idn = ac.tile([128, 128], F32R)
                nc.sync.dma_start(idn[:], id_d[:])
                onr = ac.tile([1, 64], F32R)
                nc.sync.dma_start(onr[:], onr_d[:])
                onc = ac.tile([128, 1], F32R)
                nc.sync.dma_start(onc[:], onc_d[:])
                wraw = {}
                for pname, pd in (("qq", wqq_d), ("kk", wkk_d), ("vv", wvv_d)):
                    w = ac.tile([128, C], F32, name=f"wr_{pname}", tag=f"wr_{pname}")
                    nc.sync.dma_start(w[:], pd[:])
                    wraw[pname] = w

                xh = {}  # raw x (later x1) per (b, head-half), [64, T] fp32
                for b in range(B):
                    for hl in range(2):
                        t_ = xhp.tile(
                            [64, T], F32, name=f"xh_{b}_{hl}", tag=f"xh_{b}_{hl}"
                        )
                        nc.sync.dma_start(t_[:], xown_d[b, hl])
                        xh[(b, hl)] = t_

                for b in range(B):
                    xts = []
                    for ct in range(CT):
                        t_ = xtp.tile([128, T], F32R, name="xt", tag="xt")
                        nc.sync.dma_start(t_[:], xT_d[b, ct])
                        xts.append(t_)

                    # -- LN1 stats -> s1cat/bbcat [128, 8] columns
                    s1cat = sp.tile([128, CT], F32, tag="s1cat")
                    bbcat = sp.tile([128, CT], F32, tag="bbcat")
                    for ct in range(CT):
                        st6 = sp.tile([128, 4, 6], F32, tag="st6")
                        for i in range(4):
                            nc.vector.bn_stats(
                                st6[:, i, :],
                                xts[ct][:, i * TS : (i + 1) * TS].bitcast(F32),
                            )
                        mv = sp.tile([128, 2], F32, tag="mv")
                        nc.vector.bn_aggr(mv[:], st6[:])
                        va = sp.tile([128, 1], F32, tag="va")
                        nc.vector.tensor_scalar(
                            out=va[:], in0=mv[:, 1:2], scalar1=VCORR, scalar2=EPS,
                            op0=ALU.mult, op1=ALU.add,
                        )
                        sq = sp.tile([128, 1], F32, tag="sq")
                        nc.scalar.activation(sq[:], va[:], AF.Sqrt)
                        rs = sp.tile([128, 1], F32, tag="rs")
                        with nc.allow_low_precision(reason="LN rstd"):
                            nc.vector.reciprocal(rs[:], sq[:])
                        s1 = s1cat[:, ct : ct + 1]
                        nc.vector.tensor_mul(s1, g1t[:, ct : ct + 1], rs[:])
                        bb = bbcat[:, ct : ct + 1]
                        nc.vector.tensor_mul(bb, mv[:, 0:1], s1)
                        nc.vector.tensor_sub(bb, be1t[:, ct : ct + 1], bb)

                    # -- fold LN1 into weights
                    bcol = {}
                    ws = {}
                    for pname in ("qq", "kk", "vv"):
                        bp_ = psax.tile([128, 1], F32, name="bps", tag="aux")
                        for ct in range(CT):
                            nc.tensor.matmul(
                                bp_[:], wraw[pname][:, 128 * ct : 128 * (ct + 1)],
                                bbcat[:, ct : ct + 1],
                                start=(ct == 0), stop=(ct == CT - 1),
                            )
                        bc = sp.tile([128, 1], F32, tag=f"bc_{pname}")
                        nc.vector.tensor_copy(bc[:], bp_[:])
                        bcol[pname] = bc
                        w_ = wsp.tile(
                            [128, C], F32R, name=f"ws_{pname}", tag=f"ws_{pname}"
                        )
                        nc.vector.tensor_mul(
                            w_[:].rearrange("p (c m) -> p c m", m=128),
                            wraw[pname][:].rearrange("p (c m) -> p c m", m=128),
                            s1cat[:, :, None].broadcast_to((128, CT, 128)),
                        )
                        ws[pname] = w_

                    # -- QKV streams
                    packT = {}
                    for pname in ("qq", "kk", "vv"):
                        o_ = qkp.tile(
                            [128, T], F32R, name=f"{pname}T", tag=f"{pname}T"
                        )
                        for ts in range(NTS):
                            ps = psmm.tile([128, TS], F32, name="mm", tag="mm")
                            for ct in range(CT):
                                nc.tensor.matmul(
                                    ps[:], ws[pname][:, 128 * ct : 128 * (ct + 1)],
                                    xts[ct][:, ts * TS : (ts + 1) * TS],
                                    start=(ct == 0), stop=(ct == CT - 1),
                                )
                            nc.vector.tensor_add(
                                o_[:, ts * TS : (ts + 1) * TS], ps[:],
                                bcol[pname][:].broadcast_to((128, TS)),
                            )
                        packT[pname] = o_

                    # -- V transpose to token-major packed tiles [128, 2, 65]
                    v2 = {}
                    for st in range(NST):
                        pt_ = psax.tile([128, 128], F32R, name="vT", tag="aux")
                        nc.tensor.transpose(
                            pt_[:], packT["vv"][:, st * 128 : (st + 1) * 128], idn[:]
                        )
                        v_ = vp.tile(
                            [128, 2, 65], F32R, name=f"v2_{st}", tag=f"v2_{st}"
                        )
                        nc.vector.tensor_copy(
                            v_[:, :, 0:64],
                            pt_[:].rearrange("p (h d) -> p h d", d=64),
                        )
                        nc.vector.tensor_copy(
                            v_[:, :, 64:65], onc[:, :, None].broadcast_to((128, 2, 1))
                        )
                        v2[st] = v_

                    # -- attention per head-half
                    for hl in range(2):
                        lo = 64 * hl
                        aps = [
                            psat.tile([65, TS], F32, name=f"at{ts}", tag=f"at{ts}")
                            for ts in range(NTS)
                        ]
                        for st in range(NST):
                            ts0 = st // 4
                            off = 128 * (st % 4)  # diag offset inside slice ts0
                            weis = {}
                            for ts in range(ts0, NTS):
                                w0 = off if ts == ts0 else 0
                                wid = TS - w0
                                wp = psmm.tile([128, wid], F32, name="mm", tag="mm")
                                nc.tensor.matmul(
                                    wp[:],
                                    packT["kk"][
                                        lo : lo + 64, st * 128 : (st + 1) * 128
                                    ],
                                    packT["qq"][
                                        lo : lo + 64, ts * TS + w0 : (ts + 1) * TS
                                    ],
                                    start=True, stop=True,
                                )
                                we = weip.tile([128, wid], F32R, name="wei", tag="wei")
                                nc.scalar.activation(we[:], wp[:], AF.Exp, scale=SCALE)
                                if ts == ts0:
                                    nc.vector.tensor_mul(
                                        we[:, 0:128], we[:, 0:128], tri[:]
                                    )
                                weis[ts] = (we, w0)
                            for ts in range(ts0, NTS):
                                we, w0 = weis[ts]
                                nc.tensor.matmul(
                                    aps[ts][:, w0:TS], v2[st][:, hl, :], we[:],
                                    start=(st == 0), stop=(st == 4 * ts + 3),
                                )
                            for ts in range(NTS):
                                if 4 * ts + 3 != st:
                                    continue
                                rec = sp.tile([1, TS], F32R, tag="rec")
                                with nc.allow_low_precision(reason="softmax denom"):
                                    nc.vector.reciprocal(rec[:], aps[ts][64:65, :])
                                rbp = psax.tile([64, TS], F32, name="rb", tag="aux")
                                nc.tensor.matmul(
                                    rbp[:], onr[:], rec[:], start=True, stop=True
                                )
                                rb = sp.tile([64, TS], F32, tag="rb")
                                nc.vector.tensor_copy(rb[:], rbp[:])
                                tmp = sp.tile([64, TS], F32, tag="tmp")
                                nc.vector.tensor_mul(tmp[:], aps[ts][0:64, :], rb[:])
                                xs = xh[(b, hl)][:, ts * TS : (ts + 1) * TS]
                                nc.vector.tensor_add(xs, xs, tmp[:])

                        # -- LN2 -> payload shard writes
                        st6 = sp.tile([64, 4, 6], F32, tag="st6b")
                        for i in range(4):
                            nc.vector.bn_stats(
                                st6[:, i, :], xh[(b, hl)][:, i * TS : (i + 1) * TS]
                            )
                        mv = sp.tile([64, 2], F32, tag="mv2")
                        nc.vector.bn_aggr(mv[:], st6[:])
                        va = sp.tile([64, 1], F32, tag="va2")
                        nc.vector.tensor_scalar(
                            out=va[:], in0=mv[:, 1:2], scalar1=VCORR, scalar2=EPS,
                            op0=ALU.mult, op1=ALU.add,
                        )
                        sq = sp.tile([64, 1], F32, tag="sq2")
                        nc.scalar.activation(sq[:], va[:], AF.Sqrt)
                        rs = sp.tile([64, 1], F32, tag="rs2")
                        with nc.allow_low_precision(reason="LN rstd"):
                            nc.vector.reciprocal(rs[:], sq[:])
                        sb2 = sp.tile([64, 2], F32, tag="sb2")
                        s2 = sb2[:, 0:1]
                        nc.vector.tensor_mul(s2, g2t[hl][:], rs[:])
                        b2_ = sb2[:, 1:2]
                        nc.vector.tensor_mul(b2_, mv[:, 0:1], s2)
                        nc.vector.tensor_sub(b2_, be2t[hl][:], b2_)

                        for jj in range(4):
                            j = 4 * b + jj
                            r0 = 64 * j
                            nc.sync.dma_start(
                                payl[hl][r0 : r0 + 64, 0:TS],
                                xh[(b, hl)][:, jj * TS : (jj + 1) * TS],
                            )
                            nc.sync.dma_start(payl[hl][r0 : r0 + 64, 512:514], sb2[:])

                        if b == B - 1:
                            nc.gpsimd.collective_compute(
                                "AllToAll",
                                ALU.bypass,
                                ins=[payl[hl][:]],
                                outs=[gath[hl][:]],
                                replica_groups=[list(range(NCORES))],
                            )

            # ---------------- PHASE B: MLP on own 512 tokens ----------------
            with (
                tc.tile_pool(name="bp", bufs=1) as bp,
                tc.tile_pool(name="psB", bufs=4, space="PSUM") as psB,
            ):
                x1g = []
                for i in range(CT):
                    t_ = bp.tile([128, 516], F32, name=f"x1g{i}", tag=f"x1g{i}")
                    nc.sync.dma_start(t_[0:64, :], gath[0][64 * i : 64 * (i + 1), :])
                    nc.sync.dma_start(t_[64:128, :], gath[1][64 * i : 64 * (i + 1), :])
                    x1g.append(t_)

                h1 = [
                    bp.tile([128, TS], F32R, name=f"h1_{j}", tag=f"h1_{j}")
                    for j in range(NJ)
                ]

                with tc.tile_pool(name="w1p", bufs=10) as w1p, tc.tile_pool(
                    name="y2p", bufs=1
                ) as y2p:
                    y2 = []
                    for i in range(CT):
                        t_ = y2p.tile([128, TS], F32R, name=f"y2{i}", tag=f"y2{i}")
                        nc.scalar.activation(
                            t_[:], x1g[i][:, 0:TS], AF.Identity,
                            scale=x1g[i][:, 512:513], bias=x1g[i][:, 513:514],
                        )
                        y2.append(t_)
                    for jj in range(4):
                        w1sb = []
                        for i in range(CT):
                            w_ = w1p.tile([128, 1024], F32R, name="w1", tag="w1")
                            nc.scalar.dma_start(w_[:], w1_d[i, jj])
                            w1sb.append(w_)
                        for j in range(8 * jj, 8 * jj + 8):
                            o = 128 * (j % 8)
                            ps = psB.tile([128, TS], F32, name="bm", tag="bm")
                            for i in range(CT):
                                nc.tensor.matmul(
                                    ps[:], w1sb[i][:, o : o + 128], y2[i][:],
                                    start=(i == 0), stop=(i == CT - 1),
                                )
                            nc.scalar.activation(
                                h1[j][:], ps[:], AF.Relu, bias=b1t[:, j : j + 1]
                            )

                with tc.tile_pool(name="w2p", bufs=34) as w2p:
                    for kk in range(2):
                        w2sb = []
                        for j in range(NJ):
                            w_ = w2p.tile([128, TS], F32R, name="w2", tag="w2")
                            nc.scalar.dma_start(w_[:], w2_d[j, kk])
                            w2sb.append(w_)
                        for k in range(4 * kk, 4 * kk + 4):
                            o = 128 * (k % 4)
                            ps = psB.tile([128, TS], F32, name="bm", tag="bm")
                            for j in range(NJ):
                                nc.tensor.matmul(
                                    ps[:], w2sb[j][:, o : o + 128], h1[j][:],
                                    start=(j == 0), stop=(j == NJ - 1),
                                )
                            t1 = bp.tile([128, TS], F32, name="ot", tag="ot")
                            nc.scalar.activation(
                                t1[:], ps[:], AF.Identity, bias=b2t[:, k : k + 1]
                            )
                            oo = bp.tile([128, TS], F32, name="oo", tag="oo")
                            nc.vector.tensor_add(oo[:], t1[:], x1g[k][:, 0:TS])
                            nc.sync.dma_start(outT_d[k], oo[:])

    nc.compile()
    return nc


def _prep(inputs):
    x = np.ascontiguousarray(np.asarray(inputs["x"], np.float32))
    Wq = np.asarray(inputs["Wq"], np.float32)
    Wk = np.asarray(inputs["Wk"], np.float32)
    Wv = np.asarray(inputs["Wv"], np.float32)
    W1 = np.asarray(inputs["W1"], np.float32)
    W2 = np.asarray(inputs["W2"], np.float32)
    b1 = np.asarray(inputs["b1"], np.float32)
    b2 = np.asarray(inputs["b2"], np.float32)
    g1 = np.asarray(inputs["g1"], np.float32)
    be1 = np.asarray(inputs["be1"], np.float32)
    g2 = np.asarray(inputs["g2"], np.float32)
    be2 = np.asarray(inputs["be2"], np.float32)

    xT = np.ascontiguousarray(x.reshape(B, T, CT, 128).transpose(0, 2, 3, 1))

    t_idx = np.arange(128)[None, :]
    p_idx = np.arange(128)[:, None]
    trimask = (t_idx >= p_idx).astype(np.float32)

    w1t = np.ascontiguousarray(W1.reshape(CT, 128, 4, 1024).transpose(0, 2, 1, 3))
    w2t = np.ascontiguousarray(W2.reshape(NJ, 128, 2, TS).transpose(0, 2, 1, 3))

    def packc(Wa, Wb):
        # [128, 8*128] where col block ct = rows 128ct..128ct+128 of [Wa|Wb]
        p = np.concatenate([Wa, Wb], axis=1)  # [1024, 128]
        return np.ascontiguousarray(
            p.reshape(CT, 128, 128).transpose(1, 0, 2).reshape(128, C)
        )

    shared = {
        "xT": xT,
        "g1c": np.ascontiguousarray(g1.reshape(CT, 128).T),
        "be1c": np.ascontiguousarray(be1.reshape(CT, 128).T),
        "b1c": np.ascontiguousarray(b1.reshape(NJ, 128).T),
        "b2c": np.ascontiguousarray(b2.reshape(CT, 128).T),
        "trimask": trimask,
        "identr": np.eye(128, dtype=np.float32),
        "onesrow": np.ones((1, 64), np.float32),
        "onescol": np.ones((128, 1), np.float32),
        "w1t": w1t,
        "w2t": w2t,
    }
    in_maps = []
    for c in range(NCORES):
        h0, h1_ = 2 * c, 2 * c + 1
        m = dict(shared)
        m["wqq"] = packc(Wq[h0], Wq[h1_])
        m["wkk"] = packc(Wk[h0], Wk[h1_])
        m["wvv"] = packc(Wv[h0], Wv[h1_])
        m["xown"] = np.ascontiguousarray(xT[:, c].reshape(B, 2, 64, T))
        m["g2o"] = np.ascontiguousarray(g2.reshape(CT, 2, 64)[c][:, :, None])
        m["be2o"] = np.ascontiguousarray(be2.reshape(CT, 2, 64)[c][:, :, None])
        in_maps.append(m)
    return in_maps


def kernel(**inputs) -> np.ndarray:
    if "nc" not in _CACHE:
        _CACHE["nc"] = build()
    nc = _CACHE["nc"]
    in_maps = _prep(inputs)
    res = run_bass_kernel_spmd(nc, in_maps, core_ids=list(range(NCORES)))
    out = np.empty((B, T, C), np.float32)
    for c in range(NCORES):
        b, t0 = c // 4, TS * (c % 4)
        oT = res.results[c]["outT"]  # [8, 128, 512]
        out[b, t0 : t0 + TS, :] = oT.transpose(2, 0, 1).reshape(TS, C)
    return out


# revision 6
# speedup vs baseline: 5.8467x; 5.2599x over previous
"""Fused transformer block (LN-over-sequence + causal MHA + LN + MLP, residuals)
for Trainium2, distributed over 8 NeuronCores.

Distribution:
  Phase A (attention): tensor-parallel over heads -- core c owns heads (2c, 2c+1)
    = output channels [128c, 128c+128). Everything is kept channel-major
    ("T" layouts: [channels, tokens]) so the sequence-dim LayerNorms reduce
    along the free axis and no transposes are needed.
    LN1 is folded into the QKV weights (scale weight rows by g1/std, add a
    bias column W.T @ b) so the 16MB x tensor is never rewritten.
    Softmax is computed transposed (weiT[s,t]) without max-subtraction
    (logits are ~N(0, 1/16), bounded far below exp overflow); the
    denominator comes from an appended ones-column in V (attn row 64).
    Causality: per s-tile only the t >= 128*st columns are computed
    (partial-width diagonal slices) + one [128,128] triangular mask.
  Re-shard: two AllToAlls (one per head-half, the first overlapped with the
    second half of attention) convert channel-sharding -> token-sharding:
    core c sends x1T[own channels, tokens of core j] (+ LN2 scale/bias
    columns) to core j.
  Phase B (MLP): token-parallel -- each core runs the full 1024->4096->1024
    MLP for its 512 tokens, channel-major, with the residual add fused.

All matmuls run in float32r (TF32-like, 4x fp32 throughput, ~1.5e-4 rel err).
"""

import numpy as np

import concourse.bass as bass
import concourse.mybir as mybir
import concourse.tile as tile
from concourse import bacc
from concourse.bass_utils import run_bass_kernel_spmd

B, T, C, H, D = 2, 2048, 1024, 16, 64
NCORES = 8
CT = C // 128  # 8 channel tiles
TS = 512  # token slice width
NTS = T // TS  # 4
NST = T // 128  # 16 s-tiles
HID = 4 * C
NJ = HID // 128  # 32 hidden tiles
EPS = 1e-5
VCORR = T / (T - 1.0)  # unbiased-variance correction (torch.var ddof=1)
SCALE = C**-0.5  # attention logit scale = 1/32

F32 = mybir.dt.float32
F32R = mybir.dt.float32r
AF = mybir.ActivationFunctionType
ALU = mybir.AluOpType

_CACHE: dict = {}


def build():
    nc = bacc.Bacc(None, target_bir_lowering=False, debug=False, num_devices=NCORES)

    def din(name, shape, dt=F32):
        return nc.dram_tensor(name, shape, dt, kind="ExternalInput").ap()

    xT_d = din("xT", [B, CT, 128, T], F32R)  # x channel-major (shared, rounded)
    xown_d = din("xown", [B, 2, 64, T], F32)  # core's own channels, raw fp32
    # packs laid out [128, 8*128]: col block ct holds rows 128ct..128ct+128
    wqq_d = din("wqq", [128, C], F32)
    wkk_d = din("wkk", [128, C], F32)
    wvv_d = din("wvv", [128, C], F32)
    g1_d = din("g1c", [128, CT], F32)
    be1_d = din("be1c", [128, CT], F32)
    g2_d = din("g2o", [2, 64, 1], F32)
    be2_d = din("be2o", [2, 64, 1], F32)
    b1_d = din("b1c", [128, NJ], F32)
    b2_d = din("b2c", [128, CT], F32)
    tm_d = din("trimask", [128, 128], F32R)  # tri: 1 if t_local >= s_local
    id_d = din("identr", [128, 128], F32R)
    onr_d = din("onesrow", [1, 64], F32R)
    onc_d = din("onescol", [128, 1], F32R)
    w1_d = din("w1t", [CT, 4, 128, 1024], F32R)  # W1 [c-tile, jj2, p, 2x512]
    w2_d = din("w2t", [4, 8, 128, 1024], F32R)  # W2 [j-block, j, p, 1024]

    outT_d = nc.dram_tensor("outT", [CT, 128, TS], F32, kind="ExternalOutput").ap()

    payl = [
        nc.dram_tensor(f"payl{hl}", [NCORES * 64, 516], F32).ap() for hl in range(2)
    ]
    gath = [
        nc.dram_tensor(f"gath{hl}", [NCORES * 64, 516], F32).ap() for hl in range(2)
    ]

    with tile.TileContext(nc) as tc:
        with tc.tile_pool(name="cst", bufs=1) as cst:
            b1t = cst.tile([128, NJ], F32)
            nc.sync.dma_start(b1t[:], b1_d[:])
            b2t = cst.tile([128, CT], F32)
            nc.sync.dma_start(b2t[:], b2_d[:])

            # ---------------- PHASE A ----------------
            with (
                tc.tile_pool(name="ac", bufs=1) as ac,
                tc.tile_pool(name="xtp", bufs=9) as xtp,
                tc.tile_pool(name="wsp", bufs=1) as wsp,
                tc.tile_pool(name="qkp", bufs=1) as qkp,
                tc.tile_pool(name="vp", bufs=1) as vp,
                tc.tile_pool(name="weip", bufs=6) as weip,
                tc.tile_pool(name="xhp", bufs=1) as xhp,
                tc.tile_pool(name="sp", bufs=2) as sp,
                tc.tile_pool(name="psmm", bufs=2, space="PSUM") as psmm,
                tc.tile_pool(name="psat", bufs=1, space="PSUM") as psat,
                tc.tile_pool(name="psax", bufs=2, space="PSUM") as psax,
            ):
                g1t = ac.tile([128, CT], F32)
                nc.sync.dma_start(g1t[:], g1_d[:])
                be1t = ac.tile([128, CT], F32)
                nc.sync.dma_start(be1t[:], be1_d[:])
                g2t = [
                    ac.tile([64, 1], F32, name=f"g2_{hl}", tag=f"g2_{hl}")
                    for hl in range(2)
                ]
                be2t = [
                    ac.tile([64, 1], F32, name=f"be2_{hl}", tag=f"be2_{hl}")
                    for hl in range(2)
                ]
                for hl in range(2):
                    nc.sync.dma_start(g2t[hl][:], g2_d[hl])
                    nc.sync.dma_start(be2t[hl][:], be2_d[hl])
                tri = ac.tile([128, 128], F32R)
                nc.sync.dma_start(tri[:], tm_d[:])
                idn = ac.tile([128, 128], F32R)
                nc.sync.dma_start(idn[:], id_d[:])
                onr = ac.tile([1, 64], F32R)
                nc.sync.dma_start(onr[:], onr_d[:])
                onc = ac.tile([128, 1], F32R)
                nc.sync.dma_start(onc[:], onc_d[:])
                wraw = {}
                for pname, pd in (("qq", wqq_d), ("kk", wkk_d), ("vv", wvv_d)):
                    w = ac.tile([128, C], F32, name=f"wr_{pname}", tag=f"wr_{pname}")
                    nc.sync.dma_start(w[:], pd[:])
                    wraw[pname] = w

                xh = {}  # raw x (later x1) per (b, head-half), [64, T] fp32
                for b in range(B):
                    xts = []
                    for ct in range(CT):
                        t_ = xtp.tile([128, T], F32R, name="xt", tag="xt")
                        nc.sync.dma_start(t_[:], xT_d[b, ct])
                        xts.append(t_)
                    for hl in range(2):
                        t_ = xhp.tile(
                            [64, T], F32, name=f"xh_{b}_{hl}", tag=f"xh_{b}_{hl}"
                        )
                        nc.sync.dma_start(t_[:], xown_d[b, hl])
                        xh[(b, hl)] = t_

                    # -- LN1 stats -> s1cat/bbcat [128, 8] columns
                    s1cat = sp.tile([128, CT], F32, tag="s1cat")
                    bbcat = sp.tile([128, CT], F32, tag="bbcat")
                    for ct in range(CT):
                        st6 = sp.tile([128, 4, 6], F32, tag="st6")
                        for i in range(4):
                            nc.vector.bn_stats(
                                st6[:, i, :],
                                xts[ct][:, i * TS : (i + 1) * TS].bitcast(F32),
                            )
                        mv = sp.tile([128, 2], F32, tag="mv")
                        nc.vector.bn_aggr(mv[:], st6[:])
                        va = sp.tile([128, 1], F32, tag="va")
                        nc.vector.tensor_scalar(
                            out=va[:], in0=mv[:, 1:2], scalar1=VCORR, scalar2=EPS,
                            op0=ALU.mult, op1=ALU.add,
                        )
                        sq = sp.tile([128, 1], F32, tag="sq")
                        nc.scalar.activation(sq[:], va[:], AF.Sqrt)
                        rs = sp.tile([128, 1], F32, tag="rs")
                        with nc.allow_low_precision(reason="LN rstd"):
                            nc.vector.reciprocal(rs[:], sq[:])
                        s1 = s1cat[:, ct : ct + 1]
                        nc.vector.tensor_mul(s1, g1t[:, ct : ct + 1], rs[:])
                        bb = bbcat[:, ct : ct + 1]
                        nc.vector.tensor_mul(bb, mv[:, 0:1], s1)
                        nc.vector.tensor_sub(bb, be1t[:, ct : ct + 1], bb)

                    # -- fold LN1 into weights
                    bcol = {}
                    ws = {}
                    for pname in ("qq", "kk", "vv"):
                        bp_ = psax.tile([128, 1], F32, name="bps", tag="aux")
                        for ct in range(CT):
                            nc.tensor.matmul(
                                bp_[:], wraw[pname][:, 128 * ct : 128 * (ct + 1)],
                                bbcat[:, ct : ct + 1],
                                start=(ct == 0), stop=(ct == CT - 1),
                            )
                        bc = sp.tile([128, 1], F32, tag=f"bc_{pname}")
                        nc.vector.tensor_copy(bc[:], bp_[:])
                        bcol[pname] = bc
                        w_ = wsp.tile(
                            [128, C], F32R, name=f"ws_{pname}", tag=f"ws_{pname}"
                        )
                        nc.vector.tensor_mul(
                            w_[:].rearrange("p (c m) -> p c m", m=128),
                            wraw[pname][:].rearrange("p (c m) -> p c m", m=128),
                            s1cat[:, :, None].broadcast_to((128, CT, 128)),
                        )
                        ws[pname] = w_

                    # -- QKV streams
                    packT = {}
                    for pname in ("qq", "kk", "vv"):
                        o_ = qkp.tile(
                            [128, T], F32R, name=f"{pname}T", tag=f"{pname}T"
                        )
                        for ts in range(NTS):
                            ps = psmm.tile([128, TS], F32, name="mm", tag="mm")
                            for ct in range(CT):
                                nc.tensor.matmul(
                                    ps[:], ws[pname][:, 128 * ct : 128 * (ct + 1)],
                                    xts[ct][:, ts * TS : (ts + 1) * TS],
                                    start=(ct == 0), stop=(ct == CT - 1),
                                )
                            nc.vector.tensor_add(
                                o_[:, ts * TS : (ts + 1) * TS], ps[:],
                                bcol[pname][:].broadcast_to((128, TS)),
                            )
                        packT[pname] = o_

                    # -- V transpose to token-major packed tiles [128, 2, 65]
                    v2 = {}
                    for st in range(NST):
                        pt_ = psax.tile([128, 128], F32R, name="vT", tag="aux")
                        nc.tensor.transpose(
                            pt_[:], packT["vv"][:, st * 128 : (st + 1) * 128], idn[:]
                        )
                        v_ = vp.tile(
                            [128, 2, 65], F32R, name=f"v2_{st}", tag=f"v2_{st}"
                        )
                        nc.vector.tensor_copy(
                            v_[:, :, 0:64],
                            pt_[:].rearrange("p (h d) -> p h d", d=64),
                        )
                        nc.vector.tensor_copy(
                            v_[:, :, 64:65], onc[:, :, None].broadcast_to((128, 2, 1))
                        )
                        v2[st] = v_

                    # -- attention per head-half
                    for hl in range(2):
                        lo = 64 * hl
                        aps = [
                            psat.tile([65, TS], F32, name=f"at{ts}", tag=f"at{ts}")
                            for ts in range(NTS)
                        ]
                        for st in range(NST):
                            ts0 = st // 4
                            off = 128 * (st % 4)  # diag offset inside slice ts0
                            weis = {}
                            for ts in range(ts0, NTS):
                                w0 = off if ts == ts0 else 0
                                wid = TS - w0
                                wp = psmm.tile([128, wid], F32, name="mm", tag="mm")
                                nc.tensor.matmul(
                                    wp[:],
                                    packT["kk"][
                                        lo : lo + 64, st * 128 : (st + 1) * 128
                                    ],
                                    packT["qq"][
                                        lo : lo + 64, ts * TS + w0 : (ts + 1) * TS
                                    ],
                                    start=True, stop=True,
                                )
                                we = weip.tile([128, wid], F32R, name="wei", tag="wei")
                                nc.scalar.activation(we[:], wp[:], AF.Exp, scale=SCALE)
                                if ts == ts0:
                                    nc.vector.tensor_mul(
                                        we[:, 0:128], we[:, 0:128], tri[:]
                                    )
                                weis[ts] = (we, w0)
                            for ts in range(ts0, NTS):
                                we, w0 = weis[ts]
                                nc.tensor.matmul(
                                    aps[ts][:, w0:TS], v2[st][:, hl, :], we[:],
                                    start=(st == 0), stop=(st == 4 * ts + 3),
                                )
                            for ts in range(NTS):
                                if 4 * ts + 3 != st:
                                    continue
                                rec = sp.tile([1, TS], F32R, tag="rec")
                                with nc.allow_low_precision(reason="softmax denom"):
                                    nc.vector.reciprocal(rec[:], aps[ts][64:65, :])
                                rbp = psax.tile([64, TS], F32, name="rb", tag="aux")
                                nc.tensor.matmul(
                                    rbp[:], onr[:], rec[:], start=True, stop=True
                                )
                                rb = sp.tile([64, TS], F32, tag="rb")
                                nc.vector.tensor_copy(rb[:], rbp[:])
                                tmp = sp.tile([64, TS], F32, tag="tmp")
                                nc.vector.tensor_mul(tmp[:], aps[ts][0:64, :], rb[:])
                                xs = xh[(b, hl)][:, ts * TS : (ts + 1) * TS]
                                nc.vector.tensor_add(xs, xs, tmp[:])

                        # -- LN2 -> payload shard writes
                        st6 = sp.tile([64, 4, 6], F32, tag="st6b")
                        for i in range(4):
                            nc.vector.bn_stats(
                                st6[:, i, :], xh[(b, hl)][:, i * TS : (i + 1) * TS]
                            )
                        mv = sp.tile([64, 2], F32, tag="mv2")
                        nc.vector.bn_aggr(mv[:], st6[:])
                        va = sp.tile([64, 1], F32, tag="va2")
                        nc.vector.tensor_scalar(
                            out=va[:], in0=mv[:, 1:2], scalar1=VCORR, scalar2=EPS,
                            op0=ALU.mult, op1=ALU.add,
                        )
                        sq = sp.tile([64, 1], F32, tag="sq2")
                        nc.scalar.activation(sq[:], va[:], AF.Sqrt)
                        rs = sp.tile([64, 1], F32, tag="rs2")
                        with nc.allow_low_precision(reason="LN rstd"):
                            nc.vector.reciprocal(rs[:], sq[:])
                        sb2 = sp.tile([64, 2], F32, tag="sb2")
                        s2 = sb2[:, 0:1]
                        nc.vector.tensor_mul(s2, g2t[hl][:], rs[:])
                        b2_ = sb2[:, 1:2]
                        nc.vector.tensor_mul(b2_, mv[:, 0:1], s2)
                        nc.vector.tensor_sub(b2_, be2t[hl][:], b2_)

                        for jj in range(4):
                            j = 4 * b + jj
                            r0 = 64 * j
                            nc.sync.dma_start(
                                payl[hl][r0 : r0 + 64, 0:TS],
                                xh[(b, hl)][:, jj * TS : (jj + 1) * TS],
                            )
                            nc.sync.dma_start(payl[hl][r0 : r0 + 64, 512:514], sb2[:])

                        if b == B - 1:
                            nc.gpsimd.collective_compute(
                                "AllToAll",
                                ALU.bypass,
                                ins=[payl[hl][:]],
                                outs=[gath[hl][:]],
                                replica_groups=[list(range(NCORES))],
                            )

            # ---------------- PHASE B: MLP on own 512 tokens ----------------
            with (
                tc.tile_pool(name="bp", bufs=1) as bp,
                tc.tile_pool(name="h1p", bufs=18) as h1p,
                tc.tile_pool(name="w1p", bufs=10) as w1p,
                tc.tile_pool(name="w2p", bufs=12) as w2p,
                tc.tile_pool(name="y2p", bufs=1) as y2p,
                tc.tile_pool(name="psB", bufs=4, space="PSUM") as psB,
            ):
                # W streams issued first on the sync queue: they only WAR-wait
                # on phase-A SBUF, so they run during the collectives.
                w1sb = {}
                for jj in range(4):
                    for i in range(CT):
                        w_ = w1p.tile([128, 1024], F32R, name="w1", tag="w1")
                        nc.sync.dma_start(w_[:], w1_d[i, jj])
                        w1sb[(i, jj)] = w_
                w2sb = {}
                for jb in range(4):
                    for j in range(8):
                        w_ = w2p.tile([128, 1024], F32R, name="w2", tag="w2")
                        nc.sync.dma_start(w_[:], w2_d[jb, j])
                        w2sb[(jb, j)] = w_

                x1g = []
                for i in range(CT):
                    t_ = bp.tile([128, 516], F32, name=f"x1g{i}", tag=f"x1g{i}")
                    nc.sync.dma_start(t_[0:64, :], gath[0][64 * i : 64 * (i + 1), :])
                    nc.sync.dma_start(t_[64:128, :], gath[1][64 * i : 64 * (i + 1), :])
                    x1g.append(t_)

                y2 = []
                for i in range(CT):
                    t_ = y2p.tile([128, TS], F32R, name=f"y2{i}", tag=f"y2{i}")
                    nc.scalar.activation(
                        t_[:], x1g[i][:, 0:TS], AF.Identity,
                        scale=x1g[i][:, 512:513], bias=x1g[i][:, 513:514],
                    )
                    y2.append(t_)

                acc = [
                    bp.tile([128, TS], F32, name=f"acc{k}", tag=f"acc{k}")
                    for k in range(CT)
                ]
                for jb in range(4):
                    h1blk = []
                    for j in range(8 * jb, 8 * jb + 8):
                        o = 128 * (j % 8)
                        ps = psB.tile([128, TS], F32, name="hm", tag="hm", bufs=2)
                        for i in range(CT):
                            nc.tensor.matmul(
                                ps[:], w1sb[(i, jb)][:, o : o + 128], y2[i][:],
                                start=(i == 0), stop=(i == CT - 1),
                            )
                        h_ = h1p.tile([128, TS], F32R, name="h1", tag="h1")
                        nc.scalar.activation(
                            h_[:], ps[:], AF.Relu, bias=b1t[:, j : j + 1]
                        )
                        h1blk.append(h_)
                    for k in range(CT):
                        ps = psB.tile([128, TS], F32, name="om", tag="om", bufs=2)
                        for jx in range(8):
                            nc.tensor.matmul(
                                ps[:],
                                w2sb[(jb, jx)][:, 128 * k : 128 * (k + 1)],
                                h1blk[jx][:],
                                start=(jx == 0), stop=(jx == 7),
                            )
                        if jb == 0:
                            nc.scalar.activation(
                                acc[k][:], ps[:], AF.Identity, bias=b2t[:, k : k + 1]
                            )
                        else:
                            nc.vector.tensor_add(acc[k][:], acc[k][:], ps[:])
                for k in range(CT):
                    oo = bp.tile([128, TS], F32, name="oo", tag="oo")
                    nc.vector.tensor_add(oo[:], acc[k][:], x1g[k][:, 0:TS])
                    nc.sync.dma_start(outT_d[k], oo[:])

    nc.compile()
    return nc


def _prep(inputs):
    x = np.ascontiguousarray(np.asarray(inputs["x"], np.float32))
    Wq = np.asarray(inputs["Wq"], np.float32)
    Wk = np.asarray(inputs["Wk"], np.float32)
    Wv = np.asarray(inputs["Wv"], np.float32)
    W1 = np.asarray(inputs["W1"], np.float32)
    W2 = np.asarray(inputs["W2"], np.float32)
    b1 = np.asarray(inputs["b1"], np.float32)
    b2 = np.asarray(inputs["b2"], np.float32)
    g1 = np.asarray(inputs["g1"], np.float32)
    be1 = np.asarray(inputs["be1"], np.float32)
    g2 = np.asarray(inputs["g2"], np.float32)
    be2 = np.asarray(inputs["be2"], np.float32)

    xT = np.ascontiguousarray(x.reshape(B, T, CT, 128).transpose(0, 2, 3, 1))

    t_idx = np.arange(128)[None, :]
    p_idx = np.arange(128)[:, None]
    trimask = (t_idx >= p_idx).astype(np.float32)

    w1t = np.ascontiguousarray(W1.reshape(CT, 128, 4, 1024).transpose(0, 2, 1, 3))
    w2t = np.ascontiguousarray(W2.reshape(4, 8, 128, 1024))

    def packc(Wa, Wb):
        # [128, 8*128] where col block ct = rows 128ct..128ct+128 of [Wa|Wb]
        p = np.concatenate([Wa, Wb], axis=1)  # [1024, 128]
        return np.ascontiguousarray(
            p.reshape(CT, 128, 128).transpose(1, 0, 2).reshape(128, C)
        )

    shared = {
        "xT": xT,
        "g1c": np.ascontiguousarray(g1.reshape(CT, 128).T),
        "be1c": np.ascontiguousarray(be1.reshape(CT, 128).T),
        "b1c": np.ascontiguousarray(b1.reshape(NJ, 128).T),
        "b2c": np.ascontiguousarray(b2.reshape(CT, 128).T),
        "trimask": trimask,
        "identr": np.eye(128, dtype=np.float32),
        "onesrow": np.ones((1, 64), np.float32),
        "onescol": np.ones((128, 1), np.float32),
        "w1t": w1t,
        "w2t": w2t,
    }
    in_maps = []
    for c in range(NCORES):
        h0, h1_ = 2 * c, 2 * c + 1
        m = dict(shared)
        m["wqq"] = packc(Wq[h0], Wq[h1_])
        m["wkk"] = packc(Wk[h0], Wk[h1_])
        m["wvv"] = packc(Wv[h0], Wv[h1_])
        m["xown"] = np.ascontiguousarray(xT[:, c].reshape(B, 2, 64, T))
        m["g2o"] = np.ascontiguousarray(g2.reshape(CT, 2, 64)[c][:, :, None])
        m["be2o"] = np.ascontiguousarray(be2.reshape(CT, 2, 64)[c][:, :, None])
        in_maps.append(m)
    return in_maps


def kernel(**inputs) -> np.ndarray:
    if "nc" not in _CACHE:
        _CACHE["nc"] = build()
    nc = _CACHE["nc"]
    in_maps = _prep(inputs)
    res = run_bass_kernel_spmd(nc, in_maps, core_ids=list(range(NCORES)))
    out = np.empty((B, T, C), np.float32)
    for c in range(NCORES):
        b, t0 = c // 4, TS * (c % 4)
        oT = res.results[c]["outT"]  # [8, 128, 512]
        out[b, t0 : t0 + TS, :] = oT.transpose(2, 0, 1).reshape(TS, C)
    return out
